# revision 15
# baseline (speedup 1.0000x reference)
"""Trainium2 Bass kernel for DecoupledSOLOHead mask decoding + Matrix NMS (v3).

Math (reference):
    mask_x = seg_preds_x[x_inds]; mask_y = seg_preds_y[y_inds]   # [N,H,W]
    soft = mask_x*mask_y; hard = soft > THR
    sum_masks = hard.sum((1,2)); seg_score = (soft*hard).sum((1,2))/max(sm,1)
    scores = cate_scores * seg_score
    inter = hard_flat @ hard_flat.T          # [N,N]
    ... matrix NMS (gaussian) -> scores * decay_coef

Strategy (8 cores), v3:
  - Shard H*W=60800 pixels: 7600 px/core, zero-padded to 7680 = 60 chunks
    of 128.  Gather candidate masks pixel-major via one-hot matmuls in
    bf16 (no hi/lo split; ~4e-3 soft rel err flips ~1e-4 of threshold
    decisions -> ~4e-4 on sums, inside the 2e-2 gate).
  - Candidates HOST-SORTED by class label into G=4 groups of <=128 whole
    labels.  Matrix NMS only couples same-label pairs, so the S (inter)
    matmuls stream ~128 columns instead of 500 and the AllReduce payload
    drops 250k -> 66k u16.  All 4 S accumulation regions share one PSUM
    bank; only the FIRST matmul may set start=True (start clears
    has_written for the whole bank).
  - DVE chain per chunk: soft = bf16(gxs)*gy (PSUM-capped 1x), then
    pair-batched [128,1024] hard = (soft>THR) (4x packed) and
    shs = soft*hard (2x packed TT, replacing the 1x STT).
  - One-pair software pipelining keeps the PE busy under the DVE chain.
  - Tail avoids ALL DRAM bounces (each SBUF->DRAM->SBUF round trip costs
    ~12us in DMA completion latency here):
      column->row: out[1,n] = matmul(lhsT=col[n,1], rhs=eye_f32[:n,:n])
      row->all-partitions: K=1 matmul with a [1,128] ones stationary.
    Vectors live in a group-padded [1, 128*G] row space; the host strips
    the padding and inverts the label sort.
"""

import sys

if "/opt/trn_rl_repo" not in sys.path:
    sys.path.insert(0, "/opt/trn_rl_repo")

from contextlib import ExitStack

import numpy as np
import ml_dtypes

import bass_rust
import concourse.bass as bass
import concourse.tile as tile
from concourse import bacc, mybir
from concourse.bass_utils import run_bass_kernel_spmd

N = 500
G_GRID = 128
H, W = 200, 304
HW = H * W              # 60800
NCORES = 8
PPC = HW // NCORES      # 7600 pixels per core
PAD = 7680              # padded to 60 chunks of 128
CHUNKS = PAD // 128     # 60
NPAIRS = CHUNKS // 2    # 30
THR = 0.005
SIGMA = 2.0

BF16 = mybir.dt.bfloat16
F32 = mybir.dt.float32
U16 = mybir.dt.uint16
ALU = mybir.AluOpType
AFT = bass_rust.ActivationFunctionType

_NC_CACHE = {}
_GROUPS = None   # set by _prep_inputs: (ngs tuple, perm array)


def _r2(ap, f):
    return ap.rearrange("(p f) -> p f", f=f)


def _pack_groups(labels):
    """Sort candidates by label; first-fit-decreasing whole-label blocks
    into 4 groups of <=128.  Returns (perm, ngs)."""
    labels = np.asarray(labels).astype(np.int64)
    blocks = {}
    for lab in np.unique(labels):
        blocks[int(lab)] = np.nonzero(labels == lab)[0]
    order = sorted(blocks, key=lambda k: -len(blocks[k]))
    ngroups = 4
    while True:
        bins = [[] for _ in range(ngroups)]
        fill = [0] * ngroups
        ok = True
        for lab in order:
            n = len(blocks[lab])
            placed = False
            for b in range(ngroups):
                if fill[b] + n <= 128:
                    bins[b].append(lab)
                    fill[b] += n
                    placed = True
                    break
            if not placed:
                ok = False
                break
        if ok:
            break
        ngroups += 1
    assert ngroups == 4, f"label packing needs {ngroups} groups"
    perm = np.concatenate([blocks[lab] for b in bins for lab in sorted(b)])
    ngs = tuple(sum(len(blocks[lab]) for lab in b) for b in bins if b)
    assert sum(ngs) == len(labels)
    return perm, ngs


def _build_nc(ngs):
    G = len(ngs)
    gstarts = [0]
    for n in ngs:
        gstarts.append(gstarts[-1] + n)
    # Weight slices run into the 12 pad columns of each 512-half for the
    # last group (gstart3 <= 384 always since n0+n1+n2 <= 384): junk weight
    # columns only produce junk output PARTITIONS beyond n_g, never read.
    wstarts = list(gstarts[:G])
    roffs = [0] * G
    SW = 128 * G                      # padded row width / S tile free width
    CC_NUM = 128 * SW                 # offset of num in cc buffer
    CC_LEN = CC_NUM + N

    nc = bacc.Bacc("TRN2", target_bir_lowering=False, debug=False,
                   num_devices=NCORES)

    xslab_d = nc.dram_tensor("xslab", [G_GRID, PAD], BF16, kind="ExternalInput")
    yslab_d = nc.dram_tensor("yslab", [G_GRID, PAD], BF16, kind="ExternalInput")
    ohx_d = nc.dram_tensor("ohx", [G_GRID, N], BF16, kind="ExternalInput")
    ohy_d = nc.dram_tensor("ohy", [G_GRID, N], BF16, kind="ExternalInput")
    # maskt[g][j,i] = (labels equal) & (orig_idx[i] < orig_idx[j]), padded 128
    maskt_d = nc.dram_tensor("maskt", [G, 128, 128], BF16, kind="ExternalInput")
    eye_d = nc.dram_tensor("eye", [128, 128], F32, kind="ExternalInput")
    cate_d = nc.dram_tensor("cate", [1, SW], F32, kind="ExternalInput")
    out_d = nc.dram_tensor("out", [1, SW], F32, kind="ExternalOutput")

    with tile.TileContext(nc) as tc, ExitStack() as ctx:
        consts = ctx.enter_context(tc.tile_pool(name="consts", bufs=1))
        work = ctx.enter_context(tc.tile_pool(name="work", bufs=2))
        fin = ctx.enter_context(tc.tile_pool(name="fin", bufs=1))
        psS = ctx.enter_context(tc.tile_pool(name="psS", bufs=1, space="PSUM"))
        psG = ctx.enter_context(tc.tile_pool(name="psG", bufs=1, space="PSUM"))
        dram = ctx.enter_context(tc.tile_pool(name="dram", bufs=1, space="DRAM"))

        # ---- one-hots first (gather needs them), slabs piece-major on the
        # sync queue; tail-only consts go via the gpsimd queue ----
        ohx_s = consts.tile([G_GRID, N], BF16)
        nc.sync.dma_start(ohx_s[:], ohx_d[:])
        ohy_s = consts.tile([G_GRID, N], BF16)
        nc.sync.dma_start(ohy_s[:], ohy_d[:])
        maskt_s = []
        for g in range(G):
            mt_ = consts.tile([ngs[g], ngs[g]], BF16, name=f"maskt{g}")
            nc.gpsimd.dma_start(mt_[:], maskt_d[g][:ngs[g], :ngs[g]])
            maskt_s.append(mt_)
        eye_s = consts.tile([128, 128], F32)
        nc.gpsimd.dma_start(eye_s[:], eye_d[:])
        cate_s = consts.tile([1, SW], F32)
        nc.gpsimd.dma_start(cate_s[:], cate_d[:])
        ones_s = consts.tile([G_GRID, 1], BF16)
        nc.vector.memset(ones_s[:], 1.0)
        onesrow = consts.tile([1, 128], F32)
        nc.vector.memset(onesrow[:], 1.0)

        xslab_s = consts.tile([G_GRID, PAD], BF16)
        yslab_s = consts.tile([G_GRID, PAD], BF16)
        NP = 15
        PW = PAD // NP
        for pc in range(NP):
            sl = np.s_[:, pc * PW:(pc + 1) * PW]
            nc.sync.dma_start(xslab_s[sl], xslab_d[sl])
            nc.sync.dma_start(yslab_s[sl], yslab_d[sl])

        # ---- PSUM: g pairs 2x2 (psG) + S 1 + num 1 (psS) = 6 banks ----
        s_ps = psS.tile([128, SW], F32, name="s_ps")
        num_ps = psS.tile([1, N], F32)

        LOG2 = 0.6931471805599453        # ln 2 (exp scale)
        LTHR = float(np.log2(THR))       # log2 threshold

        # ---- chunk loop, one-pair software pipelining ----
        # The slabs hold log2 of the masks, so the x*y product becomes a
        # PSUM-accumulated SUM of two one-hot gathers; soft = exp2 on the
        # scalar engine, hard = (glxy > log2 THR) on the DVE.
        pend = []
        for p in range(NPAIRS + 2):
            if p < NPAIRS:
                gp = psG.tile([128, 1024], F32, tag="g", bufs=3, name="gp")
                for h in (0, 1):
                    c = 2 * p + h
                    cs = np.s_[:, c * 128:(c + 1) * 128]
                    ho = 512 * h
                    nc.tensor.matmul(gp[:, ho:ho + N], xslab_s[cs], ohx_s[:],
                                     start=True, stop=False)
                    nc.tensor.matmul(gp[:, ho:ho + N], yslab_s[cs], ohy_s[:],
                                     start=False, stop=True)
                soft = work.tile([128, 1024], BF16, tag="soft", name="soft")
                nc.scalar.activation(soft[:], gp[:], AFT.Exp, scale=LOG2)
                hard = work.tile([128, 1024], BF16, tag="hard", name="hard")
                nc.vector.tensor_scalar(hard[:], soft[:], THR, None,
                                        op0=ALU.is_gt)
                shs = work.tile([128, 1024], BF16, tag="shs", name="shs")
                nc.vector.tensor_tensor(shs[:], soft[:], hard[:], op=ALU.mult)
                pend.append((p, shs, hard))
            if p >= 2:
                pp, shs_, hard_ = pend.pop(0)
                for h in (0, 1):
                    c = 2 * pp + h
                    first, last = (c == 0), (c == CHUNKS - 1)
                    ho = 512 * h
                    for g in range(G):
                        # start=True clears has_written for the WHOLE bank,
                        # so only the first matmul into the shared S bank
                        # may set it.
                        nc.tensor.matmul(
                            s_ps[:, 128 * g:128 * g + ngs[g]],
                            hard_[:, ho + wstarts[g]:ho + wstarts[g] + 128],
                            hard_[:, ho + gstarts[g]:ho + gstarts[g] + ngs[g]],
                            start=(first and g == 0), stop=last)
                    nc.tensor.matmul(num_ps[:], ones_s[:],
                                     shs_[:, ho:ho + N], start=first,
                                     stop=last)

        # ---- epilogue: S -> u16 SBUF, num round -> u16, pack cc ----
        s16 = fin.tile([128, SW], U16, name="s16")
        nc.vector.tensor_copy(s16[:], s_ps[:])
        numr_f = fin.tile([1, N], F32)
        nc.vector.tensor_scalar(numr_f[:], num_ps[:], 0.5, None, op0=ALU.add)
        num16 = fin.tile([1, N], U16)
        nc.vector.tensor_copy(num16[:], numr_f[:])

        cc_in = dram.tile([CC_LEN], U16)
        cc_out = dram.tile([CC_LEN], U16, addr_space="Shared")
        nc.sync.dma_start(_r2(cc_in[0:CC_NUM], SW), s16[:])
        nc.sync.dma_start(_r2(cc_in[CC_NUM:CC_NUM + N], N), num16[:])

        nc.gpsimd.collective_compute(
            "AllReduce", ALU.add, replica_groups=[list(range(NCORES))],
            ins=[cc_in.opt()], outs=[cc_out.opt()])

        # ---- post-CC: unpack, sm = diag(S), scores, decay (no bounces) ----
        st_full = fin.tile([128, SW], U16, name="st_full")
        nc.sync.dma_start(st_full[:], _r2(cc_out[0:CC_NUM], SW))
        numr = fin.tile([1, N], U16)
        nc.sync.dma_start(numr[:], _r2(cc_out[CC_NUM:CC_NUM + N], N))

        def stv(g):
            n = ngs[g]
            return st_full[roffs[g]:roffs[g] + n, 128 * g:128 * g + n]

        # sm columns per group: diag via identity mask + free-dim reduce
        smc = []
        for g in range(G):
            n = ngs[g]
            dsel = work.tile([n, n], F32, tag="dsel", name="dsel")
            nc.vector.tensor_tensor(dsel[:], stv(g), eye_s[:n, :n],
                                    op=ALU.mult)
            c = fin.tile([n, 1], F32, name=f"smc{g}")
            nc.vector.tensor_reduce(c[:], dsel[:], axis=mybir.AxisListType.X,
                                    op=ALU.add)
            smc.append(c)
        # column -> padded row via tiny matmuls (no DRAM bounce)
        smrow_t = psG.tile([128, 1024], F32, tag="g", bufs=3, name="smrow_t")
        sm_row = smrow_t[0:1, 0:SW]
        for g in range(G):
            n = ngs[g]
            nc.tensor.matmul(sm_row[:, 128 * g:128 * g + n], smc[g][:],
                             eye_s[:n, :n], start=(g == 0), stop=True)
        smrow_sb = fin.tile([1, SW], F32)
        nc.vector.tensor_copy(smrow_sb[:], sm_row[:])
        # row -> all partitions via K=1 ones matmul
        smb_t = psG.tile([128, 1024], F32, tag="g", bufs=3, name="smb_t")
        nc.tensor.matmul(smb_t[:, 0:SW], onesrow[:], smrow_sb[:],
                         start=True, stop=True)

        # scores row = cate * num / max(sm, 1)   (padded row space;
        # cate folded into the numpad re-layout copies)
        numpad = fin.tile([1, SW], F32)
        for g in range(G):
            n = ngs[g]
            nc.vector.tensor_tensor(numpad[:, 128 * g:128 * g + n],
                                    numr[:, gstarts[g]:gstarts[g] + n],
                                    cate_s[:, 128 * g:128 * g + n],
                                    op=ALU.mult)
        smx = fin.tile([1, SW], F32)
        nc.vector.tensor_scalar(smx[:], smrow_sb[:], 1.0, None, op0=ALU.max)
        rs = fin.tile([1, SW], F32)
        nc.vector.reciprocal_approx_fast(rs[:], smx[:])
        scores = fin.tile([1, SW], F32)
        nc.vector.tensor_tensor(scores[:], numpad[:], rs[:], op=ALU.mult)

        # decay per group
        csq = []
        dmt = []
        for g in range(G):
            n = ngs[g]
            u = work.tile([n, n], F32, tag="u", name="u")
            nc.vector.scalar_tensor_tensor(
                u[:], smb_t[0:n, 128 * g:128 * g + n], smc[g][:], stv(g),
                op0=ALU.add, op1=ALU.subtract)
            ru = work.tile([n, n], F32, tag="ru", name="ru")
            nc.vector.reciprocal_approx_fast(ru[:], u[:])
            iou = work.tile([n, n], F32, tag="iou", name="iou")
            nc.vector.tensor_tensor(iou[:], stv(g), ru[:], op=ALU.mult)
            sq = work.tile([n, n], F32, tag="sq", name="sq")
            nc.scalar.activation(sq[:], iou[:], AFT.Square)
            sqm = work.tile([n, n], F32, tag="sqm", name="sqm")
            nc.vector.tensor_tensor(sqm[:], sq[:], maskt_s[g][:], op=ALU.mult)
            cs_ = fin.tile([n, 1], F32, name=f"csq{g}")
            nc.vector.tensor_reduce(cs_[:], sqm[:], axis=mybir.AxisListType.X,
                                    op=ALU.max)
            csq.append(cs_)
            dm = fin.tile([n, n], F32, name=f"dm{g}")
            nc.scalar.activation(dm[:], sqm[:], AFT.Exp, scale=float(-SIGMA))
            dmt.append(dm)

        csqrow_t = psG.tile([128, 1024], F32, tag="g", bufs=3, name="csqrow_t")
        csq_row = csqrow_t[0:1, 0:SW]
        for g in range(G):
            n = ngs[g]
            nc.tensor.matmul(csq_row[:, 128 * g:128 * g + n], csq[g][:],
                             eye_s[:n, :n], start=(g == 0), stop=True)
        # 1/comp_matrix = exp(+SIGMA*comp^2), straight off PSUM
        rcmrow = fin.tile([1, SW], F32)
        nc.scalar.activation(rcmrow[:], csq_row[:], AFT.Exp,
                             scale=float(SIGMA))
        rcb_t = psG.tile([128, 1024], F32, tag="g", bufs=3, name="rcb_t")
        nc.tensor.matmul(rcb_t[:, 0:SW], onesrow[:], rcmrow[:],
                         start=True, stop=True)

        dec = []
        for g in range(G):
            n = ngs[g]
            ratio = work.tile([n, n], F32, tag="ratio", name="ratio")
            nc.vector.tensor_tensor(ratio[:], dmt[g][:],
                                    rcb_t[0:n, 128 * g:128 * g + n],
                                    op=ALU.mult)
            d = fin.tile([n, 1], F32, name=f"dec{g}")
            nc.vector.tensor_reduce(d[:], ratio[:], axis=mybir.AxisListType.X,
                                    op=ALU.min)
            dec.append(d)
        decrow_t = psG.tile([128, 1024], F32, tag="g", bufs=3, name="decrow_t")
        dec_row = decrow_t[0:1, 0:SW]
        for g in range(G):
            n = ngs[g]
            nc.tensor.matmul(dec_row[:, 128 * g:128 * g + n], dec[g][:],
                             eye_s[:n, :n], start=(g == 0), stop=True)
        res = fin.tile([1, SW], F32)
        nc.vector.tensor_tensor(res[:], scores[:], dec_row[:], op=ALU.mult)
        nc.sync.dma_start(out_d[:], res[:])

    nc.compile()
    return nc


def _get_nc():
    ngs = _GROUPS[0]
    if ngs not in _NC_CACHE:
        _NC_CACHE[ngs] = _build_nc(ngs)
    return _NC_CACHE[ngs]


def _prep_inputs(cate_scores, seg_preds_x, seg_preds_y, cate_labels, x_inds,
                 y_inds):
    global _GROUPS
    bf16 = ml_dtypes.bfloat16
    # slabs are shipped as log2(mask) so the on-device x*y product becomes
    # a PSUM-accumulated sum of the two one-hot gathers
    X = np.asarray(seg_preds_x, np.float32).reshape(G_GRID, HW)
    Y = np.asarray(seg_preds_y, np.float32).reshape(G_GRID, HW)
    X = np.maximum(np.log2(np.maximum(X, 1e-38)), -126.0).astype(bf16)
    Y = np.maximum(np.log2(np.maximum(Y, 1e-38)), -126.0).astype(bf16)

    lab = np.asarray(cate_labels).astype(np.int64)
    perm, ngs = _pack_groups(lab)
    _GROUPS = (ngs, perm)
    G = len(ngs)
    SW = 128 * G

    xi = np.asarray(x_inds).astype(np.int64)[perm]
    yi = np.asarray(y_inds).astype(np.int64)[perm]
    labp = lab[perm]
    ohx = (np.arange(G_GRID)[:, None] == xi[None, :]).astype(bf16)
    ohy = (np.arange(G_GRID)[:, None] == yi[None, :]).astype(bf16)

    gstarts = np.concatenate([[0], np.cumsum(ngs)]).astype(np.int64)
    maskt = np.zeros((G, 128, 128), bf16)
    catepad = np.zeros((1, SW), np.float32)
    catep = np.asarray(cate_scores, np.float32)[perm]
    for g in range(G):
        sl = np.s_[gstarts[g]:gstarts[g + 1]]
        pg, lg = perm[sl], labp[sl]
        m = (lg[None, :] == lg[:, None]) & (pg[None, :] < pg[:, None])
        maskt[g, :ngs[g], :ngs[g]] = m.astype(bf16)
        catepad[0, 128 * g:128 * g + ngs[g]] = catep[sl]
    eye = np.eye(128, dtype=np.float32)

    in_maps = []
    for k in range(NCORES):
        sl = np.s_[:, k * PPC:(k + 1) * PPC]
        m = {}
        for name, arr in (("xslab", X), ("yslab", Y)):
            # pads at log2 ~ -inf so padded pixels never pass the threshold
            s = np.full((G_GRID, PAD), -126.0, bf16)
            s[:, :PPC] = arr[sl]
            m[name] = s
        m["ohx"] = ohx
        m["ohy"] = ohy
        m["maskt"] = maskt
        m["eye"] = eye
        m["cate"] = catepad
        in_maps.append(m)
    return in_maps


def _postprocess(res):
    ngs, perm = _GROUPS
    arr = np.asarray(res.results[0]["out"], np.float32).reshape(-1)
    out_sorted = np.empty(N, np.float32)
    gs = 0
    for g, n in enumerate(ngs):
        out_sorted[gs:gs + n] = arr[128 * g:128 * g + n]
        gs += n
    out = np.empty(N, np.float32)
    out[perm] = out_sorted
    return out


def kernel(**inputs) -> np.ndarray:
    in_maps = _prep_inputs(**inputs)
    nc = _get_nc()
    res = run_bass_kernel_spmd(nc, in_maps, core_ids=list(range(NCORES)))
    return _postprocess(res)


if __name__ == "__main__":
    rng = np.random.default_rng(0)
    inputs = dict(
        cate_scores=rng.random(N, np.float32),
        seg_preds_x=rng.random((G_GRID, H, W), np.float32),
        seg_preds_y=rng.random((G_GRID, H, W), np.float32),
        cate_labels=rng.integers(0, 80, N),
        x_inds=rng.integers(0, G_GRID, N),
        y_inds=rng.integers(0, G_GRID, N),
    )
    out = kernel(**inputs)
    print(out[:10])
